# revision 4
# baseline (speedup 1.0000x reference)
"""DynamicGate MoE routing kernel for Trainium2 (8 NeuronCores, Bass/Tile).

Computes, for x[N,H], sim_matrix[H,E], gates[E]:
    logits = l2norm_rows(x) @ l2norm_cols(sim_matrix)
    thr    = sigmoid(gates)
    gated  = relu(logits - thr)
    mask   = (gated > 0), with top-1 fallback for all-inactive tokens
    probs  = softmax over active experts of gated
Returns (mask, probs, logits), all [N, E] fp32.

Sharding: data-parallel on the token dim across 8 cores (2048 tokens per
core); sim_matrix/gates replicated. No collectives.

v7 dataflow: x split host-side into bf16 hi/lo (x = hi + lo) in a
supertile-chunked layout; supertile sizes [512,512,512,256,256] (small
tail). Per chunk, TWO bf16 matmuls against stacked [whi|wlo] weights
produce all four split terms (exact logits); a ones-column matmul over
DVE-squared hi accumulates row sumsq. Per supertile boundary: a DVE add
combines the two PSUM halves into SBUF (freeing PSUM immediately), and
the rest of the post-processing (pair-matrix back-transpose, norms,
epilogue, output DMA) is deferred into the next supertile's chunk
stream so the PE pipeline never stalls on boundary semaphores.
"""

import sys

if "/opt/trn_rl_repo" not in sys.path:
    sys.path.insert(0, "/opt/trn_rl_repo")

import numpy as np
import ml_dtypes

import concourse.bacc as bacc
import concourse.mybir as mybir
from concourse import bass_utils
from concourse.tile import TileContext

F32 = mybir.dt.float32
BF16 = mybir.dt.bfloat16
OP = mybir.AluOpType
AF = mybir.ActivationFunctionType
AX = mybir.AxisListType

N, H, E = 16384, 2048, 64
NCORES = 8
NLOC = N // NCORES   # 2048 tokens per core
HC = H // 128        # 16 h-chunks
SUPER = [512, 512, 512, 384, 128]   # supertile token counts
NST = len(SUPER)
OFFS = [sum(SUPER[:i]) for i in range(NST)]
MAXT = max(SUPER)
EPS = 1e-12
EPS2 = EPS * EPS

PREF = 28                # chunk prefetch depth
DEFER_AT = 5             # emit deferred boundary work after this chunk


def build():
    nc = bacc.Bacc("TRN2", target_bir_lowering=False, debug=False)
    xhl_d = nc.dram_tensor("xhl", [NST * HC * 128, 2 * MAXT], BF16,
                           kind="ExternalInput")
    wstk_d = nc.dram_tensor("wstk", [128, HC * 2 * E], BF16,
                            kind="ExternalInput")
    simc_d = nc.dram_tensor("simc", [128, HC * E], F32, kind="ExternalInput")
    gates_d = nc.dram_tensor("gates", [1, E], F32, kind="ExternalInput")
    pair_d = nc.dram_tensor("pair", [128, E], F32, kind="ExternalInput")
    out_d = nc.dram_tensor("out", [NLOC, 3 * E], F32, kind="ExternalOutput")

    with TileContext(nc) as tc:
        with (
            tc.tile_pool(name="const", bufs=1) as constp,
            tc.tile_pool(name="xin", bufs=PREF // 2 + 4) as xinp,
            tc.tile_pool(name="sq", bufs=3) as sqp,
            tc.tile_pool(name="sc", bufs=2) as scp,
            tc.tile_pool(name="ep", bufs=2) as epp,
            tc.tile_pool(name="grp", bufs=2) as grpp,
            tc.tile_pool(name="outp", bufs=3) as outp,
            tc.tile_pool(name="psA", bufs=2, space="PSUM") as psA,
            tc.tile_pool(name="psB", bufs=2, space="PSUM") as psB,
            tc.tile_pool(name="psS", bufs=1, space="PSUM") as psS,
            tc.tile_pool(name="psP", bufs=2, space="PSUM") as psP,
            tc.tile_pool(name="psC", bufs=1, space="PSUM") as psC,
        ):
            # ---- chunk prefetches first: start the x stream immediately --
            x_tiles = {}
            flat = [(si, c) for si in range(NST) for c in range(HC)]

            def prefetch_pair(pk):
                # one DMA per 2 chunks: halves dispatch count
                k = 2 * pk
                if k >= len(flat):
                    return
                si, c = flat[k]
                toks = SUPER[si]
                t = xinp.tile([128, 2, 2 * MAXT], BF16, name="xc", tag="xc")
                r0 = (si * HC + c) * 128
                nc.sync.dma_start(
                    out=t[:, :, 0:2 * toks],
                    in_=xhl_d.ap()[r0:r0 + 256, 0:2 * toks].rearrange(
                        "(c p) t -> p c t", p=128),
                )
                x_tiles[pk] = t

            for pk in range(PREF // 2):
                prefetch_pair(pk)

            # ---- constants -----------------------------------------------
            onesc = constp.tile([128, 1], F32, name="onesc")
            nc.gpsimd.memset(onesc, 1.0)
            onesr = constp.tile([1, 128], F32, name="onesr")
            nc.gpsimd.memset(onesr, 1.0)
            onesb = constp.tile([128, 1], BF16, name="onesb")
            nc.scalar.copy(onesb, onesc)

            simc = constp.tile([128, HC * E], F32, name="simc")
            g_row = constp.tile([1, E], F32, name="g_row")
            pair = constp.tile([128, E], F32, name="pair")
            wstack = constp.tile([128, HC, 2 * E], BF16, name="wstack")
            nc.scalar.dma_start(
                out=wstack, in_=wstk_d.ap().rearrange("p (c e) -> p c e",
                                                      e=2 * E))
            nc.scalar.dma_start(out=simc, in_=simc_d.ap())
            nc.scalar.dma_start(out=g_row, in_=gates_d.ap())
            nc.scalar.dma_start(out=pair, in_=pair_d.ap())

            # ---- wn norm + threshold preamble ----------------------------
            simsq = constp.tile([128, HC * E], BF16, name="simsq")
            nc.scalar.square(simsq, simc)
            cs_ps = psC.tile([1, E], F32, name="cs_ps", tag="pre")
            for c in range(HC):
                nc.tensor.matmul(
                    cs_ps, lhsT=onesb, rhs=simsq[:, c * E:(c + 1) * E],
                    start=(c == 0), stop=(c == HC - 1),
                )
            wnorm = constp.tile([1, E], F32, name="wnorm")
            nc.scalar.sqrt(wnorm, cs_ps)
            nc.vector.tensor_scalar(
                out=wnorm, in0=wnorm, scalar1=EPS, scalar2=None, op0=OP.max
            )
            rwn = constp.tile([1, E], F32, name="rwn")
            nc.vector.reciprocal(rwn, wnorm)

            # thr = sigmoid(g) = 1/(1+exp(-g))
            eneg = constp.tile([1, E], F32, name="eneg")
            nc.scalar.activation(eneg, g_row, AF.Exp, scale=-1.0)
            nc.vector.tensor_scalar(
                out=eneg, in0=eneg, scalar1=1.0, scalar2=None, op0=OP.add
            )
            thr_row = constp.tile([1, E], F32, name="thr_row")
            nc.vector.reciprocal(thr_row, eneg)

            # broadcast [1,E] rows across 128 partitions via rank-1 matmul
            bc_ps = psC.tile([128, 2 * E], F32, name="bc_ps", tag="pre")
            nc.tensor.matmul(bc_ps[:, 0:E], lhsT=onesr, rhs=rwn,
                             start=True, stop=True)
            nc.tensor.matmul(bc_ps[:, E:2 * E], lhsT=onesr, rhs=thr_row,
                             start=True, stop=True)
            rwn_b = constp.tile([128, E], F32, name="rwn_b")
            thr_b = constp.tile([128, E], F32, name="thr_b")
            nc.scalar.copy(rwn_b, bc_ps[:, 0:E])
            nc.scalar.copy(thr_b, bc_ps[:, E:2 * E])

            # hi/lo bf16 split of raw sim comes pre-stacked from the host
            # (pure dtype/layout prep); column norms fold into pair_s
            pair_s = constp.tile([128, E], F32, name="pair_s")
            nc.vector.tensor_tensor(out=pair_s, in0=pair, in1=rwn_b,
                                    op=OP.mult)

            # ---- main loop -----------------------------------------------
            def make_part2(off, toks, sAB, ssqr_sb):
                nblk = toks // 128

                def part2():
                    # back-transpose + pair-sum: pl[:,j,:] = pairT @ sAB_j
                    pl = psP.tile([128, nblk, E], F32, name="pl", tag="pl")
                    for j in range(nblk):
                        nc.tensor.matmul(
                            pl[:, j, :], lhsT=sAB[:, j * 128:(j + 1) * 128],
                            rhs=pair_s, start=True, stop=True,
                        )
                    # ssq row -> [128,1] columns via bf16 rank-1 matmuls
                    rncol = psC.tile([128, nblk], F32, name="rncol", tag="pre")
                    for j in range(nblk):
                        nc.tensor.matmul(
                            rncol[:, j:j + 1],
                            lhsT=ssqr_sb[0:1, j * 128:(j + 1) * 128],
                            rhs=onesb[0:1, 0:1],
                            start=True, stop=True,
                        )
                    nrm = grpp.tile([128, nblk], F32, name="nrm", tag="nrm")
                    nc.vector.tensor_scalar(
                        out=nrm, in0=rncol, scalar1=EPS2, scalar2=None,
                        op0=OP.max,
                    )
                    nc.scalar.sqrt(nrm, nrm)
                    rng_t = grpp.tile([128, nblk], F32, name="rng_t", tag="rng")
                    nc.vector.reciprocal(rng_t, nrm)

                    # ---- epilogue on [128, nblk, E] tiles ----------------
                    def bce(ap2):
                        return ap2.unsqueeze(2).broadcast_to([128, nblk, E])

                    thr_bc = thr_b.unsqueeze(1).broadcast_to([128, nblk, E])

                    outg = outp.tile([128, nblk, 3 * E], F32, name="outg",
                                     tag="outg")
                    logits_v = outg[:, :, 2 * E:3 * E]
                    mask_v = outg[:, :, 0:E]
                    probs_v = outg[:, :, E:2 * E]

                    nc.vector.tensor_tensor(
                        out=logits_v, in0=pl, in1=bce(rng_t), op=OP.mult,
                    )
                    gsub = epp.tile([128, nblk, E], F32, name="gsub", tag="gsub")
                    nc.vector.tensor_tensor(
                        out=gsub, in0=logits_v, in1=thr_bc, op=OP.subtract,
                    )
                    gated = epp.tile([128, nblk, E], F32, name="gated",
                                     tag="gated")
                    nc.scalar.activation(gated, gsub, AF.Relu)
                    ind = epp.tile([128, nblk, E], F32, name="ind", tag="ind")
                    nc.vector.tensor_scalar(
                        out=ind, in0=gsub, scalar1=0.0, scalar2=None,
                        op0=OP.is_gt,
                    )
                    nact = grpp.tile([128, nblk], F32, name="nact", tag="nact")
                    nc.vector.tensor_reduce(
                        out=nact, in_=ind, axis=AX.X, op=OP.add,
                    )
                    inact = grpp.tile([128, nblk], F32, name="inact",
                                      tag="inact")
                    nc.vector.tensor_scalar(
                        out=inact, in0=nact, scalar1=0.0, scalar2=None,
                        op0=OP.is_equal,
                    )
                    lmax = grpp.tile([128, nblk], F32, name="lmax", tag="lmax")
                    nc.vector.tensor_reduce(
                        out=lmax, in_=logits_v, axis=AX.X, op=OP.max,
                    )
                    onehot = epp.tile([128, nblk, E], F32, name="onehot",
                                      tag="onehot")
                    nc.vector.tensor_tensor(
                        out=onehot, in0=logits_v, in1=bce(lmax),
                        op=OP.is_equal,
                    )
                    nc.vector.tensor_tensor(
                        out=mask_v, in0=onehot, in1=bce(inact), op=OP.mult,
                    )
                    nc.vector.tensor_tensor(
                        out=mask_v, in0=mask_v, in1=ind, op=OP.add,
                    )
                    # masked softmax of gated (values in [0,2]: exp safe; the
                    # reference's max-subtraction cancels in the ratio)
                    ex = epp.tile([128, nblk, E], F32, name="ex", tag="ex")
                    nc.scalar.activation(ex, gated, AF.Exp)
                    me = epp.tile([128, nblk, E], F32, name="me", tag="me")
                    nc.vector.tensor_tensor(
                        out=me, in0=ex, in1=mask_v, op=OP.mult,
                    )
                    sesum = grpp.tile([128, nblk], F32, name="sesum",
                                      tag="sesum")
                    nc.vector.tensor_reduce(
                        out=sesum, in_=me, axis=AX.X, op=OP.add,
                    )
                    rs = grpp.tile([128, nblk], F32, name="rs", tag="rs")
                    nc.vector.reciprocal(rs, sesum)
                    nc.vector.tensor_tensor(
                        out=probs_v, in0=me, in1=bce(rs), op=OP.mult,
                    )

                    nc.scalar.dma_start(
                        out=out_d.ap()[off:off + toks, :].rearrange(
                            "(j p) e -> p j e", p=128),
                        in_=outg,
                    )

                return part2

            pending = None
            for si in range(NST):
                toks = SUPER[si]
                off = OFFS[si]
                pA = psA.tile([128, MAXT], F32, name="pA", tag="pA")
                pB = psB.tile([128, MAXT], F32, name="pB", tag="pB")
                ssqrow = psS.tile([1, MAXT], F32, name="ssqrow", tag="ssq")

                for c in range(HC):
                    k = si * HC + c
                    tp = x_tiles[k // 2]
                    if c % 2 == 1:
                        x_tiles.pop(k // 2)
                        prefetch_pair(k // 2 + PREF // 2)
                    hi = tp[:, c % 2, 0:toks]
                    lo = tp[:, c % 2, toks:2 * toks]
                    ws_c = wstack[:, c, :]

                    nc.tensor.matmul(pA[:, 0:toks], lhsT=ws_c, rhs=hi,
                                     start=(c == 0), stop=(c == HC - 1))
                    nc.tensor.matmul(pB[:, 0:toks], lhsT=ws_c, rhs=lo,
                                     start=(c == 0), stop=(c == HC - 1))

                    xsq = sqp.tile([128, MAXT], BF16, name="xsq", tag="xsq")
                    nc.vector.tensor_tensor(out=xsq[:, 0:toks], in0=hi,
                                            in1=hi, op=OP.mult)
                    if c % 2 == 0:
                        xsq_hold = xsq
                    else:
                        # sum over h = sum over (chunk, partition): pairwise
                        # DVE add halves the PE colsum stream; bf16 rounding
                        # of positive sums is a per-token scale (argmax-safe)
                        xsq2 = sqp.tile([128, MAXT], BF16, name="xsq2",
                                        tag="xsq2")
                        nc.vector.tensor_tensor(
                            out=xsq2[:, 0:toks], in0=xsq_hold[:, 0:toks],
                            in1=xsq[:, 0:toks], op=OP.add,
                        )
                        nc.tensor.matmul(
                            ssqrow[0:1, 0:toks], lhsT=onesb,
                            rhs=xsq2[:, 0:toks],
                            start=(c == 1), stop=(c == HC - 1),
                        )

                    if c == DEFER_AT and pending is not None:
                        pending()
                        pending = None

                # part 1: free PSUM fast — combine halves to SBUF, copy ssq
                # (DVE may read only one PSUM input: stage pA via ACT first)
                sAB = scp.tile([128, MAXT], F32, name="sAB", tag="sAB")
                nc.scalar.copy(sAB[:, 0:toks], pA[:, 0:toks])
                nc.vector.tensor_tensor(
                    out=sAB[:, 0:toks], in0=sAB[:, 0:toks], in1=pB[:, 0:toks],
                    op=OP.add,
                )
                ssqr_sb = scp.tile([1, MAXT], BF16, name="ssqr_sb", tag="ssqr")
                nc.scalar.copy(ssqr_sb[0:1, 0:toks], ssqrow[0:1, 0:toks])

                pending = make_part2(off, toks, sAB[:, 0:toks],
                                     ssqr_sb[0:1, 0:toks])

            pending()

    nc.compile()
    return nc


_NC_CACHE = {}


def _get_nc():
    if "nc" not in _NC_CACHE:
        _NC_CACHE["nc"] = build()
    return _NC_CACHE["nc"]


def make_in_maps(x, sim_matrix, gates):
    x = np.ascontiguousarray(np.asarray(x, dtype=np.float32))
    sim = np.ascontiguousarray(np.asarray(sim_matrix, dtype=np.float32))
    g = np.ascontiguousarray(np.asarray(gates, dtype=np.float32)).reshape(1, E)
    simc = np.ascontiguousarray(
        sim.reshape(HC, 128, E).transpose(1, 0, 2).reshape(128, HC * E)
    )
    pair = np.ascontiguousarray(np.tile(np.eye(E, dtype=np.float32), (2, 1)))
    whi = simc.astype(ml_dtypes.bfloat16)
    wlo = (simc - whi.astype(np.float32)).astype(ml_dtypes.bfloat16)
    # [128, HC*E] pair -> [128, HC, 2E] stacked
    wstk = np.concatenate(
        [whi.reshape(128, HC, E), wlo.reshape(128, HC, E)], axis=2
    ).reshape(128, HC * 2 * E)
    wstk = np.ascontiguousarray(wstk)
    maps = []
    for cc in range(NCORES):
        xl = x[cc * NLOC:(cc + 1) * NLOC]
        xhl = np.zeros((NST * HC * 128, 2 * MAXT), dtype=ml_dtypes.bfloat16)
        for si in range(NST):
            toks = SUPER[si]
            off = OFFS[si]
            # [toks, H] -> [c, p, t]
            xt = np.ascontiguousarray(
                xl[off:off + toks].reshape(toks, HC, 128).transpose(1, 2, 0)
            )
            hi = xt.astype(ml_dtypes.bfloat16)
            lo = (xt - hi.astype(np.float32)).astype(ml_dtypes.bfloat16)
            blk = np.concatenate([hi, lo], axis=-1)  # [c, p, 2*toks]
            xhl[si * HC * 128:(si + 1) * HC * 128, 0:2 * toks] = (
                blk.reshape(HC * 128, 2 * toks)
            )
        maps.append({"xhl": xhl, "simc": simc, "gates": g, "pair": pair,
                     "wstk": wstk})
    return maps


def kernel(x, sim_matrix, gates):
    nc = _get_nc()
    in_maps = make_in_maps(x, sim_matrix, gates)
    res = bass_utils.run_bass_kernel_spmd(nc, in_maps, core_ids=list(range(NCORES)))
    outs = [res.results[c]["out"] for c in range(NCORES)]
    full = np.concatenate(outs, axis=0)
    mask = np.ascontiguousarray(full[:, 0:E])
    probs = np.ascontiguousarray(full[:, E:2 * E])
    logits = np.ascontiguousarray(full[:, 2 * E:3 * E])
    return mask, probs, logits
